# revision 1
# baseline (speedup 1.0000x reference)
"""Trainium2 Bass kernel for nn_ConvAttention (sparse_attention).

Reference computes, per batch b and query position i (along L):
    qkv = W1 @ x (1x1 conv);  Q,K,V split
    S[b,i,j] = conv5x5(Q[b,i] + K[b,j]) + b2
    attn     = softmax_j(S)
    out[b,i] = sum_j attn[b,i,j] * V[b,j]

Key algebra (exact, not approximate):
  * conv is linear => conv(Q_i + K_j) = conv(Q_i) + conv(K_j).
  * conv(Q_i), b2, and the conv of K's bias b1k are all constant along the
    softmax axis j, so they cancel in softmax exactly.
  * Therefore attn is independent of i, and out[b,:, :, :, i] is the same for
    every i:  out = sum_j softmax_j(conv(W1k @ x_j)) * (W1v @ x_j + b1v).
  * The 1x1 K-projection folds into the 5x5 conv weights on the host:
    W2eff[o,c,dy,dx] = sum_k W2[o,k,dy,dx] * W1k[k,c].

Device work per core (H sharded, 2 output rows + 2-row halo per core):
  * V projection: 2 matmuls (128x128 @ 128x512).
  * Score conv: 25 taps x 2 row-banks of matmuls (K=128 via block-diagonal
    batch packing: partitions = b*64+c), accumulated in PSUM.
  * softmax over l (innermost, 32 contiguous) + weighted V sum on ACT/DVE.
Host: pad/shard x, fold weights, gather 16KB/core outputs, add b1v,
broadcast over l.
"""

import numpy as np

B, C, H, W, L = 2, 64, 16, 16, 32
NCORES = 8
RPC = H // NCORES          # output rows per core (2)
HALO = RPC + 4             # input rows held per core (6)
WPAD = W + 4               # zero-padded width (20)
P = 2 * C                  # partitions: b*64 + channel

_PLAN = None


class _Plan:
    def __init__(self):
        import concourse.bacc as bacc
        import concourse.tile as tile
        from concourse import mybir

        f32 = mybir.dt.float32
        nc = bacc.Bacc("TRN2", target_bir_lowering=False, debug=False,
                       num_devices=NCORES)

        xh_d = nc.dram_tensor("xh", [P, HALO, WPAD, L], f32, kind="ExternalInput")
        w2_d = nc.dram_tensor("w2", [P, 25, P], f32, kind="ExternalInput")
        xv_d = nc.dram_tensor("xv", [P, RPC, W, L], f32, kind="ExternalInput")
        wv_d = nc.dram_tensor("wv", [P, P], f32, kind="ExternalInput")
        o_d = nc.dram_tensor("o", [P, RPC, W], f32, kind="ExternalOutput")

        taps = [(dy, dx) for dy in range(-2, 3) for dx in range(-2, 3)]

        with tile.TileContext(nc) as tc:
            with (
                tc.tile_pool(name="sb", bufs=1) as sb,
                tc.tile_pool(name="work", bufs=2) as work,
                tc.tile_pool(name="psum", bufs=2, space="PSUM") as psum,
            ):
                wv_t = sb.tile([P, P], f32, tag="wv")
                nc.sync.dma_start(out=wv_t[:], in_=wv_d[:])
                xv_t = sb.tile([P, RPC, W, L], f32, tag="xv")
                nc.sync.dma_start(out=xv_t[:], in_=xv_d[:])

                xrow = []
                for i in range(HALO):
                    t = sb.tile([P, WPAD, L], f32, tag=f"xh{i}")
                    nc.sync.dma_start(out=t[:], in_=xh_d[:, i])
                    xrow.append(t)

                w2c = []
                for i in range(5):
                    t = sb.tile([P, 5, P], f32, tag=f"w2{i}")
                    nc.sync.dma_start(out=t[:], in_=w2_d[:, 5 * i:5 * (i + 1), :])
                    w2c.append(t)

                # V projection, one PSUM bank per output row r
                v_s = []
                for r in range(RPC):
                    vp = psum.tile([P, W * L], f32, tag="vp")
                    nc.tensor.matmul(vp[:], lhsT=wv_t[:], rhs=xv_t[:, r],
                                     start=True, stop=True)
                    vs = sb.tile([P, W, L], f32, tag=f"v{r}")
                    nc.scalar.copy(vs[:], vp[:])
                    v_s.append(vs)

                for r in range(RPC):
                    ck = psum.tile([P, W * L], f32, tag="ck")
                    for ti, (dy, dx) in enumerate(taps):
                        nc.tensor.matmul(
                            ck[:],
                            lhsT=w2c[ti // 5][:, ti % 5, :],
                            rhs=xrow[2 + r + dy][:, 2 + dx:2 + dx + W, :],
                            start=(ti == 0),
                            stop=(ti == len(taps) - 1),
                        )
                    e = work.tile([P, W, L], f32, tag="e")
                    nc.scalar.activation(e[:], ck[:],
                                         func=mybir.ActivationFunctionType.Exp)
                    s = work.tile([P, W], f32, tag="s")
                    nc.vector.tensor_reduce(out=s[:], in_=e[:],
                                            axis=mybir.AxisListType.X,
                                            op=mybir.AluOpType.add)
                    rcp = work.tile([P, W], f32, tag="rcp")
                    nc.vector.reciprocal(rcp[:], s[:])
                    tt = work.tile([P, W, L], f32, tag="tt")
                    nc.vector.tensor_mul(tt[:], e[:], v_s[r][:])
                    u = work.tile([P, W], f32, tag="u")
                    nc.vector.tensor_reduce(out=u[:], in_=tt[:],
                                            axis=mybir.AxisListType.X,
                                            op=mybir.AluOpType.add)
                    o_t = work.tile([P, W], f32, tag="o")
                    nc.vector.tensor_mul(o_t[:], u[:], rcp[:])
                    nc.sync.dma_start(out=o_d[:, r], in_=o_t[:])

        nc.compile()
        self.nc = nc


def _get_plan():
    global _PLAN
    if _PLAN is None:
        _PLAN = _Plan()
    return _PLAN


def _prep_in_maps(x, W1, W2):
    # Fold the K-projection into the conv weights (in float64 for accuracy).
    W1k = W1[C:2 * C, :, 0, 0].astype(np.float64)          # [k, c]
    W2eff = np.einsum("okyx,kc->ocyx", W2.astype(np.float64), W1k)
    eff = np.ascontiguousarray(
        W2eff.transpose(1, 2, 3, 0).reshape(C, 25, C)      # [c_in, tap, o]
    ).astype(np.float32)
    w2p = np.zeros((P, 25, P), np.float32)
    w2p[:C, :, :C] = eff
    w2p[C:, :, C:] = eff

    W1v = W1[2 * C:3 * C, :, 0, 0]                          # [o, c]
    wvp = np.zeros((P, P), np.float32)
    wvp[:C, :C] = W1v.T
    wvp[C:, C:] = W1v.T

    in_maps = []
    for m in range(NCORES):
        g0 = RPC * m - 2
        buf = np.zeros((B, C, HALO, WPAD, L), np.float32)
        lo, hi = max(g0, 0), min(g0 + HALO, H)
        buf[:, :, lo - g0:hi - g0, 2:2 + W, :] = x[:, :, lo:hi, :, :]
        xh = buf.reshape(P, HALO, WPAD, L)
        xv = np.ascontiguousarray(
            x[:, :, RPC * m:RPC * (m + 1), :, :]).reshape(P, RPC, W, L)
        in_maps.append({"xh": xh, "w2": w2p, "xv": xv, "wv": wvp})
    return in_maps


def kernel(x, W1, b1, W2, b2):
    from concourse.bass_utils import run_bass_kernel_spmd

    x = np.asarray(x, dtype=np.float32)
    W1 = np.asarray(W1, dtype=np.float32)
    b1 = np.asarray(b1, dtype=np.float32)
    W2 = np.asarray(W2, dtype=np.float32)

    plan = _get_plan()
    in_maps = _prep_in_maps(x, W1, W2)
    res = run_bass_kernel_spmd(plan.nc, in_maps, core_ids=list(range(NCORES)))

    b1v = b1[2 * C:3 * C].astype(np.float32)
    out = np.empty((B, C, H, W, L), np.float32)
    for m in range(NCORES):
        o = res.results[m]["o"].reshape(B, C, RPC, W)       # [b, c, r, w]
        o = o + b1v[None, :, None, None]
        out[:, :, RPC * m:RPC * (m + 1), :, :] = o[..., None]
    return out


# revision 2
# speedup vs baseline: 2.5650x; 2.5650x over previous
"""Trainium2 Bass kernel for nn_ConvAttention (sparse_attention).

Reference computes, per batch b and query position i (along L):
    qkv = W1 @ x (1x1 conv);  Q,K,V split
    S[b,i,j] = conv5x5(Q[b,i] + K[b,j]) + b2
    attn     = softmax_j(S)
    out[b,i] = sum_j attn[b,i,j] * V[b,j]

Key algebra (exact, not approximate):
  * conv is linear => conv(Q_i + K_j) = conv(Q_i) + conv(K_j).
  * conv(Q_i), b2, and the conv of K's bias b1k are all constant along the
    softmax axis j, so they cancel in softmax exactly.
  * Therefore attn is independent of i, and out[b,:, :, :, i] is the same for
    every i:  out = sum_j softmax_j(conv(W1k @ x_j)) * (W1v @ x_j + b1v).
  * The 1x1 K-projection folds into the 5x5 conv weights on the host:
    W2eff[o,c,dy,dx] = sum_k W2[o,k,dy,dx] * W1k[k,c].

Device work per core (H sharded, 2 output rows + 2-row halo per core):
  * Score conv: 25 taps x 2 row-banks of matmuls (K=128 via block-diagonal
    batch packing: partitions = b*64+c), accumulated in PSUM.
  * V projection: 2 matmuls (128x128 @ 128x512).
  * softmax over l (innermost, 32 contiguous) + weighted V sum on ACT/DVE.
Host: pad/shard x, fold weights, gather 16KB/core outputs, add b1v,
broadcast over l.
"""

import os

import numpy as np

B, C, H, W, L = 2, 64, 16, 16, 32
NCORES = 8
RPC = H // NCORES          # output rows per core (2)
HALO = RPC + 4             # input rows held per core (6)
WPAD = W + 4               # zero-padded width (20)
P = 2 * C                  # partitions: b*64 + channel

CONV_DT = os.environ.get("CONV_DT", "f32r")   # f32 | f32r | bf16
V_DT = os.environ.get("V_DT", "f32r")
N_WARMUP = int(os.environ.get("N_WARMUP", "6"))

_PLAN = None


def _mm_dt(mybir, name):
    return {"f32": mybir.dt.float32,
            "f32r": mybir.dt.float32r,
            "bf16": mybir.dt.bfloat16}[name]


def _np_dt(name):
    if name == "bf16":
        import ml_dtypes
        return ml_dtypes.bfloat16
    return np.float32


class _Plan:
    def __init__(self):
        import concourse.bacc as bacc
        import concourse.tile as tile
        from concourse import mybir

        f32 = mybir.dt.float32
        cdt = _mm_dt(mybir, CONV_DT)
        vdt = _mm_dt(mybir, V_DT)
        bf16 = mybir.dt.bfloat16
        nc = bacc.Bacc("TRN2", target_bir_lowering=False, debug=False,
                       num_devices=NCORES)

        xh_d = nc.dram_tensor("xh", [P, HALO, WPAD, L], cdt, kind="ExternalInput")
        w2_d = nc.dram_tensor("w2", [P, 25, P], cdt, kind="ExternalInput")
        xv_d = nc.dram_tensor("xv", [P, RPC, W, L], vdt, kind="ExternalInput")
        wv_d = nc.dram_tensor("wv", [P, P], vdt, kind="ExternalInput")
        o_d = nc.dram_tensor("o", [P, RPC, W], f32, kind="ExternalOutput")

        taps = [(dy, dx) for dy in range(-2, 3) for dx in range(-2, 3)]

        with tile.TileContext(nc) as tc:
            with (
                tc.tile_pool(name="sb", bufs=1) as sb,
                tc.tile_pool(name="work", bufs=2) as work,
                tc.tile_pool(name="psum", bufs=2, space="PSUM") as psum,
            ):
                # PE warmup during input DMA: keeps HAM from starting the conv
                # cold. Dummy bf16 matmuls on memset tiles (no DMA deps).
                if N_WARMUP:
                    wdum = sb.tile([P, P], bf16, tag="wdum")
                    rdum = sb.tile([P, 512], bf16, tag="rdum")
                    nc.gpsimd.memset(wdum[:], 0)
                    nc.gpsimd.memset(rdum[:], 0)
                    pdum = psum.tile([P, 512], f32, tag="pdum")
                    for i in range(N_WARMUP):
                        nc.tensor.matmul(pdum[:], lhsT=wdum[:], rhs=rdum[:],
                                         start=(i == 0), stop=(i == N_WARMUP - 1))

                # input DMAs, in the order the conv consumes them
                xrow = []
                for i in range(HALO):
                    t = sb.tile([P, WPAD, L], cdt, tag=f"xh{i}")
                    nc.sync.dma_start(out=t[:], in_=xh_d[:, i])
                    xrow.append(t)
                w2c = []
                for i in range(5):
                    t = sb.tile([P, 5, P], cdt, tag=f"w2{i}")
                    nc.sync.dma_start(out=t[:], in_=w2_d[:, 5 * i:5 * (i + 1), :])
                    w2c.append(t)
                wv_t = sb.tile([P, P], vdt, tag="wv")
                nc.sync.dma_start(out=wv_t[:], in_=wv_d[:])
                xv_t = sb.tile([P, RPC, W, L], vdt, tag="xv")
                nc.sync.dma_start(out=xv_t[:], in_=xv_d[:])

                # score conv: accumulate 25 taps into one PSUM bank per row
                cks = []
                for r in range(RPC):
                    ck = psum.tile([P, W * L], f32, tag="ck")
                    for ti, (dy, dx) in enumerate(taps):
                        nc.tensor.matmul(
                            ck[:],
                            lhsT=w2c[ti // 5][:, ti % 5, :],
                            rhs=xrow[2 + r + dy][:, 2 + dx:2 + dx + W, :],
                            start=(ti == 0),
                            stop=(ti == len(taps) - 1),
                        )
                    cks.append(ck)

                # V projection (after conv in PE order; xv arrives during conv)
                vps = []
                for r in range(RPC):
                    vp = psum.tile([P, W * L], f32, tag="vp")
                    nc.tensor.matmul(vp[:], lhsT=wv_t[:], rhs=xv_t[:, r],
                                     start=True, stop=True)
                    vps.append(vp)

                # softmax over l + weighted V sum
                for r in range(RPC):
                    e = work.tile([P, W, L], f32, tag="e")
                    nc.scalar.activation(e[:], cks[r][:],
                                         func=mybir.ActivationFunctionType.Exp)
                    s = work.tile([P, W], f32, tag="s")
                    nc.vector.tensor_reduce(out=s[:], in_=e[:],
                                            axis=mybir.AxisListType.X,
                                            op=mybir.AluOpType.add)
                    rcp = work.tile([P, W], f32, tag="rcp")
                    nc.vector.reciprocal(rcp[:], s[:])
                    tt = work.tile([P, W, L], f32, tag="tt")
                    nc.vector.tensor_mul(tt[:], e[:],
                                         vps[r][:].rearrange("p (w l) -> p w l", l=L))
                    u = work.tile([P, W], f32, tag="u")
                    nc.vector.tensor_reduce(out=u[:], in_=tt[:],
                                            axis=mybir.AxisListType.X,
                                            op=mybir.AluOpType.add)
                    o_t = work.tile([P, W], f32, tag="o")
                    nc.vector.tensor_mul(o_t[:], u[:], rcp[:])
                    nc.sync.dma_start(out=o_d[:, r], in_=o_t[:])

        nc.compile()
        self.nc = nc


def _get_plan():
    global _PLAN
    if _PLAN is None:
        _PLAN = _Plan()
    return _PLAN


def _prep_in_maps(x, W1, W2):
    cnp = _np_dt(CONV_DT)
    vnp = _np_dt(V_DT)

    # Fold the K-projection into the conv weights (in float64 for accuracy).
    W1k = W1[C:2 * C, :, 0, 0].astype(np.float64)          # [k, c]
    W2eff = np.einsum("okyx,kc->ocyx", W2.astype(np.float64), W1k)
    eff = np.ascontiguousarray(
        W2eff.transpose(1, 2, 3, 0).reshape(C, 25, C)      # [c_in, tap, o]
    ).astype(np.float32)
    w2p = np.zeros((P, 25, P), np.float32)
    w2p[:C, :, :C] = eff
    w2p[C:, :, C:] = eff
    w2p = w2p.astype(cnp)

    W1v = W1[2 * C:3 * C, :, 0, 0]                          # [o, c]
    wvp = np.zeros((P, P), np.float32)
    wvp[:C, :C] = W1v.T
    wvp[C:, C:] = W1v.T
    wvp = wvp.astype(vnp)

    in_maps = []
    for m in range(NCORES):
        g0 = RPC * m - 2
        buf = np.zeros((B, C, HALO, WPAD, L), np.float32)
        lo, hi = max(g0, 0), min(g0 + HALO, H)
        buf[:, :, lo - g0:hi - g0, 2:2 + W, :] = x[:, :, lo:hi, :, :]
        xh = buf.reshape(P, HALO, WPAD, L).astype(cnp)
        xv = np.ascontiguousarray(
            x[:, :, RPC * m:RPC * (m + 1), :, :]).reshape(P, RPC, W, L).astype(vnp)
        in_maps.append({"xh": xh, "w2": w2p, "xv": xv, "wv": wvp})
    return in_maps


def kernel(x, W1, b1, W2, b2):
    from concourse.bass_utils import run_bass_kernel_spmd

    x = np.asarray(x, dtype=np.float32)
    W1 = np.asarray(W1, dtype=np.float32)
    b1 = np.asarray(b1, dtype=np.float32)
    W2 = np.asarray(W2, dtype=np.float32)

    plan = _get_plan()
    in_maps = _prep_in_maps(x, W1, W2)
    res = run_bass_kernel_spmd(plan.nc, in_maps, core_ids=list(range(NCORES)))

    b1v = b1[2 * C:3 * C].astype(np.float32)
    out = np.empty((B, C, H, W, L), np.float32)
    for m in range(NCORES):
        o = res.results[m]["o"].reshape(B, C, RPC, W)       # [b, c, r, w]
        o = o + b1v[None, :, None, None]
        out[:, :, RPC * m:RPC * (m + 1), :, :] = o[..., None]
    return out
